# revision 12
# baseline (speedup 1.0000x reference)
"""AxiomGraphNN (GNN message passing) Trainium2 Bass kernel, 8-way SPMD.

Sharding: target nodes i (rows of states / columns of the NxN score
tensors) are sharded across 8 cores; parameters are replicated; per-layer
messages m = states @ msg_W and attention source terms a_nb are computed
by each node's owner core and AllGathered so every core can contract over
the full source axis j.

Per-core score/aggregation pipeline per (layer, edge):
  q[j,i,h] = (a_cur[i,h]+b[h] + a_nb[j,h]) * wmask[j,i]     (DVE fused STT)
  u = leaky_relu(q, 0.2)                                    (ACT Prelu)
  p~ = exp(u)                                               (ACT Exp, 4 heads batched)
  p = (wmask==0) - p~        == -(masked softmax numerator) (DVE fused STT)
  agg_unnorm[i,(d|1)] = sum_j p[j,i] * [m[j,h,:] | 1]       (PE, p-stationary,
                         ones column gives the softmax denominator for free;
                         the global negation cancels in the normalization)
  x = agg/colsum + msg_b                                    (DVE reciprocal + STT)
GRU runs in the transposed [d,n] orientation (matmul friendly), LayerNorm in
[n,d]; rsqrt via Newton iteration on DVE (no ACT table switch).
"""

import numpy as np

import concourse.bacc as bacc
import concourse.bass as bass
import concourse.mybir as mybir
import concourse.tile as tile
from concourse import bass_utils, masks

F32 = mybir.dt.float32
A = mybir.AluOpType
AF = mybir.ActivationFunctionType

N, D, DH, H, E, L = 2048, 256, 512, 4, 2, 3
DHEAD = DH // H
LN_EPS = 1e-5
NCORES = 8
NI = N // NCORES          # 256 local target nodes
NT = NI // 128            # 2 i-tiles
JT = N // 128             # 16 j-tiles
DT = D // 128             # 2 d-tiles
KT = 3 * D // 128         # 6 gate k-tiles
CT = E * DH // 128        # 8 x-contraction tiles
AGW = E * DH + 2 * H      # AllGather row width: m (1024) + a_nb (8)

_STATE: dict = {}


def _newton_rsqrt(nc, sb, v, n_free, tag_prefix):
    """rstd = 1/sqrt(v) on DVE via bit-trick seed + 3 Newton iterations.

    v: [128, n_free] SBUF fp32 AP (overwritten is fine). Returns a tile AP.
    """
    I32 = mybir.dt.int32
    y = sb.tile([128, n_free], F32, tag=f"{tag_prefix}_y", bufs=2, name=f"{tag_prefix}_y")
    t1 = sb.tile([128, n_free], F32, tag=f"{tag_prefix}_t1", bufs=2, name=f"{tag_prefix}_t1")
    t2 = sb.tile([128, n_free], F32, tag=f"{tag_prefix}_t2", bufs=2, name=f"{tag_prefix}_t2")
    nc.vector.tensor_scalar(y.bitcast(I32)[:], v.bitcast(I32), 1, None, op0=A.logical_shift_right)
    nc.vector.tensor_scalar(y.bitcast(I32)[:], y.bitcast(I32)[:], -1, 0x5F3759DF, op0=A.mult, op1=A.add)
    for _ in range(3):
        nc.vector.tensor_tensor(t1[:], y[:], y[:], op=A.mult)
        nc.vector.tensor_tensor(t2[:], t1[:], v, op=A.mult)
        nc.vector.tensor_scalar(t2[:], t2[:], -0.5, 1.5, op0=A.mult, op1=A.add)
        nc.vector.tensor_tensor(y[:], y[:], t2[:], op=A.mult)
    return y


def build_kernel(apply_ln_affine: bool, ob2_val: float, dump: bool = False):
    nc = bacc.Bacc("TRN2", target_bir_lowering=False, debug=False, num_devices=NCORES)
    dumps = {}

    def mkdump(name, shape):
        dumps[name] = nc.dram_tensor(f"dbg_{name}", shape, F32, kind="ExternalOutput")
        return dumps[name]

    # ---- kernel I/O ----
    axT = nc.dram_tensor("axT", [D, NI], F32, kind="ExternalInput")
    wmask = nc.dram_tensor("wmask", [E, N, NI], F32, kind="ExternalInput")
    ia = nc.dram_tensor("ia", [NI, 1], F32, kind="ExternalInput")
    inW = nc.dram_tensor("inW", [D, D], F32, kind="ExternalInput")
    inb = nc.dram_tensor("inb", [1, D], F32, kind="ExternalInput")
    attWc = nc.dram_tensor("attWc", [L, D, E * H], F32, kind="ExternalInput")
    attWn = nc.dram_tensor("attWn", [L, D, E * H], F32, kind="ExternalInput")
    attb = nc.dram_tensor("attb", [L, E * H, 1], F32, kind="ExternalInput")
    msgW = nc.dram_tensor("msgW", [L, E, D, DH], F32, kind="ExternalInput")
    msgb = nc.dram_tensor("msgb", [L, 1, E * DH], F32, kind="ExternalInput")
    WihT = nc.dram_tensor("WihT", [L, E * DH, 3 * D], F32, kind="ExternalInput")
    bih = nc.dram_tensor("bih", [L, 3 * D, 1], F32, kind="ExternalInput")
    WhhT = nc.dram_tensor("WhhT", [L, D, 3 * D], F32, kind="ExternalInput")
    bhh = nc.dram_tensor("bhh", [L, 3 * D, 1], F32, kind="ExternalInput")
    lng = nc.dram_tensor("lng", [L, 1, D], F32, kind="ExternalInput")
    lnb = nc.dram_tensor("lnb", [L, 1, D], F32, kind="ExternalInput")
    oW1 = nc.dram_tensor("oW1", [D, DH], F32, kind="ExternalInput")
    ob1 = nc.dram_tensor("ob1", [DH, 1], F32, kind="ExternalInput")
    oW2 = nc.dram_tensor("oW2", [DH, 1], F32, kind="ExternalInput")
    states_out = nc.dram_tensor("states_out", [NI, D], F32, kind="ExternalOutput")
    act_out = nc.dram_tensor("act_out", [NI, 1], F32, kind="ExternalOutput")

    with tile.TileContext(nc) as tc:
        with (
            tc.tile_pool(name="res", bufs=1) as res,        # resident, whole-kernel
            tc.tile_pool(name="parms", bufs=1) as parms,    # per-layer params
            tc.tile_pool(name="wk", bufs=2) as wk,          # working tiles
            tc.tile_pool(name="ps", bufs=2, space="PSUM") as ps,
            tc.tile_pool(name="dram", bufs=2, space="DRAM") as dram,
        ):
            # ---- resident loads ----
            w_sb = res.tile([128, E, JT, NI], F32)
            nc.sync.dma_start(w_sb[:], wmask.ap().rearrange("e (jt p) i -> p e jt i", p=128))
            axT_sb = res.tile([128, DT, NI], F32)
            nc.sync.dma_start(axT_sb[:], axT.ap().rearrange("(dt p) n -> p dt n", p=128))
            ia_sb = res.tile([128, NT, 1], F32)
            nc.sync.dma_start(ia_sb[:], ia.ap().rearrange("(nt p) o -> p nt o", p=128))
            ident = res.tile([128, 128], F32)
            masks.make_identity(nc, ident[:])
            ones1 = res.tile([1, 128], F32)
            nc.vector.memset(ones1[:], 1.0)

            def pe_bcast_row(row_ap, width, out_sb_ap):
                """Broadcast a [1, width] SBUF row across 128 partitions via PE."""
                bc_ps = ps.tile([128, 512], F32, tag="bc", bufs=2, name="bc_ps")
                nc.tensor.matmul(bc_ps[:, 0:width], ones1[:], row_ap, start=True, stop=True)
                nc.vector.tensor_copy(out_sb_ap, bc_ps[:, 0:width])

            def transpose128(in_ap, out_ap):
                tp = ps.tile([128, 128], F32, tag="tp", bufs=2, name="tp_ps")
                nc.tensor.transpose(tp[:], in_ap, ident[:])
                nc.vector.tensor_copy(out_ap, tp[:])

            # ---- input transform: states0 = (ax @ inW + inb) * ia ----
            inW_sb = res.tile([128, DT, D], F32)
            nc.sync.dma_start(inW_sb[:], inW.ap().rearrange("(dt p) d -> p dt d", p=128))
            inb_row = res.tile([1, D], F32)
            nc.sync.dma_start(inb_row[:], inb[:, :])
            inb_bc = res.tile([128, D], F32)
            pe_bcast_row(inb_row[:], D, inb_bc[:])

            states_sb = wk.tile([128, NT, D], F32, tag="states", bufs=2, name="states0")
            for nt in range(NT):
                s_ps = ps.tile([128, 512], F32, tag="bc", bufs=2, name="s0_ps")
                for dt in range(DT):
                    nc.tensor.matmul(
                        s_ps[:, 0:D],
                        axT_sb[:, dt, nt * 128:(nt + 1) * 128],
                        inW_sb[:, dt, :],
                        start=(dt == 0), stop=(dt == DT - 1),
                    )
                t0 = wk.tile([128, D], F32, tag="t0", bufs=2, name="t0")
                nc.vector.tensor_tensor(t0[:], s_ps[:, 0:D], inb_bc[:], op=A.add)
                nc.vector.tensor_scalar(states_sb[:, nt, :], t0[:], ia_sb[:, nt, :], None, op0=A.mult)
            if dump:
                nc.sync.dma_start(mkdump("states0", [128, NT, D]).ap(), states_sb[:])

            # ---- layers ----
            for l in range(L):
                # per-layer params
                attWc_sb = parms.tile([128, DT, E * H], F32, tag="attWc", bufs=1, name=f"attWc{l}")
                nc.sync.dma_start(attWc_sb[:], attWc[l].rearrange("(dt p) c -> p dt c", p=128))
                attWn_sb = parms.tile([128, DT, E * H], F32, tag="attWn", bufs=1, name=f"attWn{l}")
                nc.sync.dma_start(attWn_sb[:], attWn[l].rearrange("(dt p) c -> p dt c", p=128))
                attb_sb = parms.tile([E * H, 1], F32, tag="attb", bufs=1, name=f"attb{l}")
                nc.sync.dma_start(attb_sb[:], attb[l])
                msgW_sb = parms.tile([128, E, DT, DH], F32, tag="msgW", bufs=1, name=f"msgW{l}")
                nc.sync.dma_start(msgW_sb[:], msgW[l].rearrange("e (dt p) f -> p e dt f", p=128))
                msgb_row = parms.tile([1, E * DH], F32, tag="msgb", bufs=1, name=f"msgb{l}")
                nc.sync.dma_start(msgb_row[:], msgb[l])
                WihT_sb = parms.tile([128, CT, 3 * D], F32, tag="WihT", bufs=1, name=f"WihT{l}")
                nc.sync.dma_start(WihT_sb[:], WihT[l].rearrange("(ct p) k -> p ct k", p=128))
                bih_sb = parms.tile([128, KT, 1], F32, tag="bih", bufs=1, name=f"bih{l}")
                nc.sync.dma_start(bih_sb[:], bih[l].rearrange("(kt p) o -> p kt o", p=128))
                WhhT_sb = parms.tile([128, DT, 3 * D], F32, tag="WhhT", bufs=1, name=f"WhhT{l}")
                nc.sync.dma_start(WhhT_sb[:], WhhT[l].rearrange("(dt p) k -> p dt k", p=128))
                bhh_sb = parms.tile([128, KT, 1], F32, tag="bhh", bufs=1, name=f"bhh{l}")
                nc.sync.dma_start(bhh_sb[:], bhh[l].rearrange("(kt p) o -> p kt o", p=128))

                # transpose states -> statesT [128, DT, NI]
                statesT_sb = wk.tile([128, DT, NI], F32, tag="statesT", bufs=2, name=f"statesT{l}")
                for nt in range(NT):
                    for dt in range(DT):
                        transpose128(
                            states_sb[:, nt, dt * 128:(dt + 1) * 128],
                            statesT_sb[:, dt, nt * 128:(nt + 1) * 128],
                        )

                if dump and l == 0:
                    nc.sync.dma_start(mkdump("statesT", [128, DT, NI]).ap(), statesT_sb[:])
                # a_curT [EH, NI] = attWc.T @ statesT (+ attb)
                acur_ps = ps.tile([E * H, NI], F32, tag="bc", bufs=2, name="acur_ps")
                for dt in range(DT):
                    nc.tensor.matmul(
                        acur_ps[:], attWc_sb[:, dt, :], statesT_sb[:, dt, :],
                        start=(dt == 0), stop=(dt == DT - 1),
                    )
                acurT_sb = wk.tile([E * H, NI], F32, tag="acurT", bufs=2, name=f"acurT{l}")
                nc.vector.tensor_scalar(acurT_sb[:], acur_ps[:], attb_sb[:], None, op0=A.add)
                if dump and l == 0:
                    nc.sync.dma_start(mkdump("acurT", [E * H, NI]).ap(), acurT_sb[:])

                # a_nb + m local, staged in one SBUF tile -> single ag_in writer
                pk_sb = wk.tile([128, NT, AGW], F32, tag="pk", bufs=1, name=f"pk{l}")
                for nt in range(NT):
                    anb_ps = ps.tile([128, E * H], F32, tag="tp", bufs=2, name="anb_ps")
                    for dt in range(DT):
                        nc.tensor.matmul(
                            anb_ps[:], statesT_sb[:, dt, nt * 128:(nt + 1) * 128],
                            attWn_sb[:, dt, :],
                            start=(dt == 0), stop=(dt == DT - 1),
                        )
                    nc.vector.tensor_copy(pk_sb[:, nt, E * DH:], anb_ps[:])

                ag_in = dram.tile([NI, AGW], F32, tag="ag_in", bufs=2, name=f"ag_in{l}")
                for e in range(E):
                    for nt in range(NT):
                        m_ps = ps.tile([128, DH], F32, tag="bc", bufs=2, name="m_ps")
                        for dt in range(DT):
                            nc.tensor.matmul(
                                m_ps[:], statesT_sb[:, dt, nt * 128:(nt + 1) * 128],
                                msgW_sb[:, e, dt, :],
                                start=(dt == 0), stop=(dt == DT - 1),
                            )
                        nc.vector.tensor_copy(pk_sb[:, nt, e * DH:(e + 1) * DH], m_ps[:])
                nc.sync.dma_start(
                    ag_in.rearrange("(nt p) c -> p nt c", p=128), pk_sb[:]
                )

                ag_out = dram.tile([N, AGW], F32, tag="ag_out", bufs=2,
                                   addr_space="Shared", name=f"ag_out{l}")
                nc.gpsimd.collective_compute(
                    "AllGather", A.bypass,
                    replica_groups=[list(range(NCORES))],
                    ins=[ag_in.opt()], outs=[ag_out.opt()],
                )

                # unpack gathered a_nb [128, JT, EH]
                anb_sb = wk.tile([128, JT, E * H], F32, tag="anb", bufs=2, name=f"anb{l}")
                nc.sync.dma_start(
                    anb_sb[:],
                    ag_out[:, E * DH:].rearrange("(jt p) c -> p jt c", p=128),
                )
                if dump and l == 0:
                    nc.sync.dma_start(mkdump("anb", [128, JT, E * H]).ap(), anb_sb[:])
                    nc.sync.dma_start(mkdump("agm", [128, JT, DH]).ap(),
                                      ag_out[:, 0:DH].rearrange("(jt p) c -> p jt c", p=128))

                # x = concat over (e, h) of normalized aggregates [128, NT, E*DH]
                x_sb = wk.tile([128, NT, E * DH], F32, tag="x", bufs=1, name=f"x{l}")

                for e in range(E):
                    # msg_b broadcast
                    mbbc = wk.tile([128, DH], F32, tag="mbbc", bufs=1, name=f"mbbc{l}{e}")
                    pe_bcast_row(msgb_row[:, e * DH:(e + 1) * DH], DH, mbbc[:])

                    # r = a_cur + attb broadcast per head
                    rbc = wk.tile([128, H, NI], F32, tag="rbc", bufs=1, name=f"rbc{l}{e}")
                    for h in range(H):
                        arow = wk.tile([1, NI], F32, tag="arow", bufs=2, name="arow")
                        nc.sync.dma_start(arow[:], acurT_sb[e * H + h:e * H + h + 1, :])
                        pe_bcast_row(arow[:], NI, rbc[:, h, :])

                    for hp in range(2):  # head pairs -> 4 psum banks (2h x 2it), one accum group each
                        agg_ps = {
                            (h2, it): ps.tile([128, DHEAD + 1], F32, tag=f"agg{h2}{it}",
                                              bufs=1, name=f"agg{l}{e}{hp}{h2}{it}")
                            for h2 in range(2) for it in range(NT)
                        }
                        JTC = 2  # j-tiles per streamed message chunk
                        for jc in range(JT // JTC):
                            mt = wk.tile([128, JTC, 2, DHEAD + 1], F32, tag="mt", bufs=2,
                                         name=f"mt{l}{e}{hp}{jc}")
                            for h2 in range(2):
                                h = hp * 2 + h2
                                nc.sync.dma_start(
                                    mt[:, :, h2, 0:DHEAD],
                                    ag_out[jc * JTC * 128:(jc + 1) * JTC * 128,
                                           e * DH + h * DHEAD: e * DH + (h + 1) * DHEAD]
                                    .rearrange("(jt p) d -> p jt d", p=128),
                                )
                            nc.vector.memset(mt[:, :, :, DHEAD:DHEAD + 1], 1.0)
                            for jj in range(JTC):
                                jt = jc * JTC + jj
                                u4 = wk.tile([128, 2, NI], F32, tag="u4", bufs=2, name="u4")
                                for h2 in range(2):
                                    h = hp * 2 + h2
                                    q = wk.tile([128, NI], F32, tag="q", bufs=3, name="q")
                                    nc.vector.scalar_tensor_tensor(
                                        q[:], rbc[:, h, :],
                                        anb_sb[:, jt, e * H + h:e * H + h + 1],
                                        w_sb[:, e, jt, :],
                                        op0=A.add, op1=A.mult,
                                    )
                                    if dump and l == 0 and e == 0 and jt == 0 and h == 0:
                                        nc.sync.dma_start(mkdump("q00", [128, NI]).ap(), q[:])
                                    nc.scalar.activation(u4[:, h2, :], q[:], AF.Prelu, alpha=0.2)
                                p4 = wk.tile([128, 2, NI], F32, tag="p4", bufs=2, name="p4")
                                nc.scalar.activation(p4[:], u4[:], AF.Exp)
                                pn4 = wk.tile([128, 2, NI], F32, tag="pn4", bufs=2, name="pn4")
                                for h2 in range(2):
                                    nc.vector.scalar_tensor_tensor(
                                        pn4[:, h2, :], w_sb[:, e, jt, :], 0.0, p4[:, h2, :],
                                        op0=A.is_equal, op1=A.subtract,
                                    )
                                if dump and l == 0 and e == 0 and jt == 0 and hp == 0:
                                    nc.sync.dma_start(mkdump("u400", [128, 2, NI]).ap(), u4[:])
                                    nc.sync.dma_start(mkdump("p400", [128, 2, NI]).ap(), p4[:])
                                    nc.sync.dma_start(mkdump("pn400", [128, 2, NI]).ap(), pn4[:])
                                    nc.sync.dma_start(mkdump("mt00", [128, JTC, 2, DHEAD + 1]).ap(), mt[:])
                                for h2 in range(2):
                                    for it in range(NT):
                                        nc.tensor.matmul(
                                            agg_ps[(h2, it)][:],
                                            pn4[:, h2, it * 128:(it + 1) * 128],
                                            mt[:, jj, h2, :],
                                            start=(jt == 0), stop=(jt == JT - 1),
                                        )
                        # normalize + msg_b for this head pair
                        for h2 in range(2):
                            h = hp * 2 + h2
                            for it in range(NT):
                                inv = wk.tile([128, 1], F32, tag="inv", bufs=4, name="inv")
                                nc.vector.reciprocal(inv[:], agg_ps[(h2, it)][:, DHEAD:DHEAD + 1])
                                nc.vector.scalar_tensor_tensor(
                                    x_sb[:, it, e * DH + h * DHEAD: e * DH + (h + 1) * DHEAD],
                                    agg_ps[(h2, it)][:, 0:DHEAD], inv[:],
                                    mbbc[:, h * DHEAD:(h + 1) * DHEAD],
                                    op0=A.mult, op1=A.add,
                                )
                if dump and l == 0:
                    nc.sync.dma_start(mkdump("x", [128, NT, E * DH]).ap(), x_sb[:])
                # ---- GRU ----
                xT_sb = wk.tile([128, CT, NI], F32, tag="xT", bufs=1, name=f"xT{l}")
                for it in range(NT):
                    for c8 in range(CT):
                        transpose128(
                            x_sb[:, it, c8 * 128:(c8 + 1) * 128],
                            xT_sb[:, c8, it * 128:(it + 1) * 128],
                        )
                giT_sb = wk.tile([128, KT, NI], F32, tag="giT", bufs=1, name=f"giT{l}")
                for kt in range(KT):
                    g_ps = ps.tile([128, NI], F32, tag="bc", bufs=2, name="gi_ps")
                    for ct in range(CT):
                        nc.tensor.matmul(
                            g_ps[:], WihT_sb[:, ct, kt * 128:(kt + 1) * 128],
                            xT_sb[:, ct, :],
                            start=(ct == 0), stop=(ct == CT - 1),
                        )
                    nc.vector.tensor_scalar(giT_sb[:, kt, :], g_ps[:], bih_sb[:, kt, :], None, op0=A.add)
                ghT_sb = wk.tile([128, KT, NI], F32, tag="ghT", bufs=1, name=f"ghT{l}")
                for kt in range(KT):
                    g_ps = ps.tile([128, NI], F32, tag="bc", bufs=2, name="gh_ps")
                    for dt in range(DT):
                        nc.tensor.matmul(
                            g_ps[:], WhhT_sb[:, dt, kt * 128:(kt + 1) * 128],
                            statesT_sb[:, dt, :],
                            start=(dt == 0), stop=(dt == DT - 1),
                        )
                    nc.vector.tensor_scalar(ghT_sb[:, kt, :], g_ps[:], bhh_sb[:, kt, :], None, op0=A.add)

                if dump and l == 0:
                    nc.sync.dma_start(mkdump("giT", [128, KT, NI]).ap(), giT_sb[:])
                    nc.sync.dma_start(mkdump("ghT", [128, KT, NI]).ap(), ghT_sb[:])
                hT_sb = wk.tile([128, DT, NI], F32, tag="hT", bufs=2, name=f"hT{l}")
                for g2 in range(DT):
                    gsum = wk.tile([128, NI], F32, tag="gsum", bufs=3, name="gsum")
                    # r gate
                    nc.vector.tensor_tensor(gsum[:], giT_sb[:, g2, :], ghT_sb[:, g2, :], op=A.add)
                    rg = wk.tile([128, NI], F32, tag="rg", bufs=2, name="rg")
                    nc.scalar.activation(rg[:], gsum[:], AF.Tanh, scale=0.5)
                    nc.vector.tensor_scalar(rg[:], rg[:], 0.5, 0.5, op0=A.mult, op1=A.add)
                    # z gate
                    nc.vector.tensor_tensor(gsum[:], giT_sb[:, 2 + g2, :], ghT_sb[:, 2 + g2, :], op=A.add)
                    zg = wk.tile([128, NI], F32, tag="zg", bufs=2, name="zg")
                    nc.scalar.activation(zg[:], gsum[:], AF.Tanh, scale=0.5)
                    nc.vector.tensor_scalar(zg[:], zg[:], 0.5, 0.5, op0=A.mult, op1=A.add)
                    # n gate: tanh(i_n + r*h_n)
                    nc.vector.tensor_tensor(gsum[:], rg[:], ghT_sb[:, 4 + g2, :], op=A.mult)
                    nc.vector.tensor_tensor(gsum[:], gsum[:], giT_sb[:, 4 + g2, :], op=A.add)
                    ng = wk.tile([128, NI], F32, tag="ng", bufs=2, name="ng")
                    nc.scalar.activation(ng[:], gsum[:], AF.Tanh)
                    # h = n + z*(states - n)
                    nc.vector.tensor_tensor(gsum[:], statesT_sb[:, g2, :], ng[:], op=A.subtract)
                    nc.vector.tensor_tensor(gsum[:], zg[:], gsum[:], op=A.mult)
                    nc.vector.tensor_tensor(hT_sb[:, g2, :], ng[:], gsum[:], op=A.add)

                if dump and l == 0:
                    nc.sync.dma_start(mkdump("hT", [128, DT, NI]).ap(), hT_sb[:])
                # transpose back to [n, d]
                hnew_sb = wk.tile([128, NT, D], F32, tag="states", bufs=2, name=f"states{l + 1}")
                for dt in range(DT):
                    for nt in range(NT):
                        transpose128(
                            hT_sb[:, dt, nt * 128:(nt + 1) * 128],
                            hnew_sb[:, nt, dt * 128:(dt + 1) * 128],
                        )

                # ---- LayerNorm ----
                if apply_ln_affine:
                    lng_row = parms.tile([1, D], F32, tag="lngr", bufs=1, name=f"lngr{l}")
                    nc.sync.dma_start(lng_row[:], lng[l])
                    lnb_row = parms.tile([1, D], F32, tag="lnbr", bufs=1, name=f"lnbr{l}")
                    nc.sync.dma_start(lnb_row[:], lnb[l])
                    lng_bc = wk.tile([128, D], F32, tag="lngbc", bufs=2, name=f"lngbc{l}")
                    pe_bcast_row(lng_row[:], D, lng_bc[:])
                    lnb_bc = wk.tile([128, D], F32, tag="lnbbc", bufs=2, name=f"lnbbc{l}")
                    pe_bcast_row(lnb_row[:], D, lnb_bc[:])

                for nt in range(NT):
                    red = wk.tile([128, 1], F32, tag="red", bufs=2, name="red")
                    nc.vector.tensor_reduce(red[:], hnew_sb[:, nt, :], axis=mybir.AxisListType.X, op=A.add)
                    negmu = wk.tile([128, 1], F32, tag="negmu", bufs=2, name="negmu")
                    nc.vector.tensor_scalar(negmu[:], red[:], -1.0 / D, None, op0=A.mult)
                    sq = wk.tile([128, D], F32, tag="sq", bufs=2, name="sq")
                    ssq = wk.tile([128, 1], F32, tag="ssq", bufs=2, name="ssq")
                    nc.scalar.activation(sq[:], hnew_sb[:, nt, :], AF.Square, accum_out=ssq[:])
                    # var = ssq/D - mu^2 ; v = var + eps
                    vv = wk.tile([128, 1], F32, tag="vv", bufs=2, name="vv")
                    nc.vector.tensor_scalar(vv[:], ssq[:], 1.0 / D, LN_EPS, op0=A.mult, op1=A.add)
                    mu2 = wk.tile([128, 1], F32, tag="mu2", bufs=2, name="mu2")
                    nc.vector.tensor_tensor(mu2[:], negmu[:], negmu[:], op=A.mult)
                    nc.vector.tensor_tensor(vv[:], vv[:], mu2[:], op=A.subtract)
                    rstd = _newton_rsqrt(nc, wk, vv[:], 1, "rs")
                    # states = (h - mu) * rstd  (then optional affine)
                    hc = wk.tile([128, D], F32, tag="hc", bufs=2, name="hc")
                    nc.vector.tensor_scalar(hc[:], hnew_sb[:, nt, :], negmu[:], None, op0=A.add)
                    nc.vector.tensor_scalar(hnew_sb[:, nt, :], hc[:], rstd[:], None, op0=A.mult)
                    if apply_ln_affine:
                        nc.vector.tensor_tensor(hnew_sb[:, nt, :], hnew_sb[:, nt, :], lng_bc[:], op=A.mult)
                        nc.vector.tensor_tensor(hnew_sb[:, nt, :], hnew_sb[:, nt, :], lnb_bc[:], op=A.add)

                if dump and l == 0:
                    nc.sync.dma_start(mkdump("states1", [128, NT, D]).ap(), hnew_sb[:])
                states_sb = hnew_sb

            # ---- output head ----
            statesT_f = wk.tile([128, DT, NI], F32, tag="statesT", bufs=2, name="statesTf")
            for nt in range(NT):
                for dt in range(DT):
                    transpose128(
                        states_sb[:, nt, dt * 128:(dt + 1) * 128],
                        statesT_f[:, dt, nt * 128:(nt + 1) * 128],
                    )
            oW1_sb = res.tile([128, DT, DH], F32)
            nc.sync.dma_start(oW1_sb[:], oW1.ap().rearrange("(dt p) f -> p dt f", p=128))
            ob1_sb = res.tile([128, H, 1], F32)
            nc.sync.dma_start(ob1_sb[:], ob1.ap().rearrange("(dht p) o -> p dht o", p=128))
            oW2_sb = res.tile([128, H, 1], F32)
            nc.sync.dma_start(oW2_sb[:], oW2.ap().rearrange("(dht p) o -> p dht o", p=128))

            h1_sb = wk.tile([128, H, NI], F32, tag="h1", bufs=1, name="h1")
            for dht in range(H):
                h_ps = ps.tile([128, NI], F32, tag="bc", bufs=2, name="h1_ps")
                for dt in range(DT):
                    nc.tensor.matmul(
                        h_ps[:], oW1_sb[:, dt, dht * 128:(dht + 1) * 128],
                        statesT_f[:, dt, :],
                        start=(dt == 0), stop=(dt == DT - 1),
                    )
                nc.scalar.activation(h1_sb[:, dht, :], h_ps[:], AF.Relu, bias=ob1_sb[:, dht, :])
            act_sb = wk.tile([128, NT, 1], F32, tag="acts", bufs=1, name="acts")
            for nt in range(NT):
                a_ps = ps.tile([128, 1], F32, tag="tp", bufs=2, name="a_ps")
                for dht in range(H):
                    nc.tensor.matmul(
                        a_ps[:], h1_sb[:, dht, nt * 128:(nt + 1) * 128],
                        oW2_sb[:, dht, :],
                        start=(dht == 0), stop=(dht == H - 1),
                    )
                nc.vector.tensor_scalar(act_sb[:, nt, :], a_ps[:], ob2_val, None, op0=A.add)

            nc.sync.dma_start(states_out.ap().rearrange("(nt p) d -> p nt d", p=128), states_sb[:])
            nc.sync.dma_start(act_out.ap().rearrange("(nt p) o -> p nt o", p=128), act_sb[:])

    nc.compile()
    return nc


def _np(x):
    return np.asarray(x, dtype=np.float32)


def prepare_inputs(axiom_states, adj_implies, adj_supports, w_implies, w_supports,
                   initial_activations, params):
    ax = _np(axiom_states)
    ia = _np(initial_activations)
    adjs = [np.asarray(adj_implies), np.asarray(adj_supports)]
    ws = [_np(w_implies), _np(w_supports)]
    p = params

    # w * mask with exact-zero entries reserved for masked edges
    wmask = np.stack([
        np.where(adjs[e] != 0, np.maximum(ws[e], 1e-30), 0.0).astype(np.float32)
        for e in range(E)
    ])  # [E, N, N]

    layers = p["layers"]
    shared = {
        "inW": _np(p["input_W"]),
        "inb": _np(p["input_b"]).reshape(1, D),
        "attWc": np.stack([
            np.concatenate([_np(lp["att_Wc"][e]) for e in range(E)], axis=1)
            for lp in layers
        ]),
        "attWn": np.stack([
            np.concatenate([_np(lp["att_Wn"][e]) for e in range(E)], axis=1)
            for lp in layers
        ]),
        "attb": np.stack([
            _np(lp["att_b"]).reshape(E * H, 1) for lp in layers
        ]),
        "msgW": np.stack([_np(lp["msg_W"]) for lp in layers]),
        "msgb": np.stack([
            _np(lp["msg_b"]).reshape(1, E * DH) for lp in layers
        ]),
        "WihT": np.stack([np.ascontiguousarray(_np(lp["gru_Wih"]).T) for lp in layers]),
        "bih": np.stack([_np(lp["gru_bih"]).reshape(3 * D, 1) for lp in layers]),
        "WhhT": np.stack([np.ascontiguousarray(_np(lp["gru_Whh"]).T) for lp in layers]),
        "bhh": np.stack([_np(lp["gru_bhh"]).reshape(3 * D, 1) for lp in layers]),
        "lng": np.stack([_np(lp["ln_g"]).reshape(1, D) for lp in layers]),
        "lnb": np.stack([_np(lp["ln_b"]).reshape(1, D) for lp in layers]),
        "oW1": _np(p["out_W1"]),
        "ob1": _np(p["out_b1"]).reshape(DH, 1),
        "oW2": _np(p["out_W2"]).reshape(DH, 1),
    }
    apply_ln_affine = not all(
        np.allclose(lp["ln_g"], 1.0) and np.allclose(lp["ln_b"], 0.0) for lp in layers
    )
    ob2_val = float(np.asarray(p["out_b2"]).reshape(-1)[0])

    in_maps = []
    for k in range(NCORES):
        sh = slice(k * NI, (k + 1) * NI)
        m = dict(shared)
        m["axT"] = np.ascontiguousarray(ax[sh].T)
        m["wmask"] = np.ascontiguousarray(wmask[:, :, sh])
        m["ia"] = np.ascontiguousarray(ia[sh].reshape(NI, 1))
        in_maps.append(m)
    return in_maps, apply_ln_affine, ob2_val


def kernel(axiom_states, adj_implies, adj_supports, w_implies, w_supports,
           initial_activations, params):
    in_maps, apply_ln_affine, ob2_val = prepare_inputs(
        axiom_states, adj_implies, adj_supports, w_implies, w_supports,
        initial_activations, params)

    key = ("nc", apply_ln_affine, ob2_val)
    if key not in _STATE:
        _STATE[key] = build_kernel(apply_ln_affine, ob2_val)
    nc = _STATE[key]

    res = bass_utils.run_bass_kernel_spmd(nc, in_maps, core_ids=list(range(NCORES)))
    states = np.concatenate([res.results[c]["states_out"] for c in range(NCORES)], axis=0)
    acts = np.concatenate(
        [res.results[c]["act_out"][:, 0] for c in range(NCORES)], axis=0)
    return states, acts
